# revision 43
# baseline (speedup 1.0000x reference)
"""Trainium2 Bass kernel v4 for PhysicsInformedNN (Navier-Stokes PINN).

Jet propagation with 11 channels: val, x, y, t, xx, xy, yy, xt, yt,
gx (= psi_xxx + psi_xyy), gy (= psi_xxy + psi_yyy).  The Laplacian
contraction works because third derivatives only enter f_u/f_v via
u_xx+u_yy = d_y(Lap psi) and v_xx+v_yy = -d_x(Lap psi).

Per hidden layer, per supertile of 6x512 points:
  - PSUM: tri tiles [128,1536] (3 banks) x bufs=2 + pair [128,1024]:
    tri_a=(VAL,X,Y) tri_b=(T,XX,XY) tri_c=(YY,XT,YT) pair=(GX,GY)
  - ACT: tanh, t0^2, f1=1-t0^2, qq + batched PSUM->SBUF fp16 copies
  - DVE: wide fp16 2x-mode tensor_tensor with stride-0 broadcast APs;
    29 product-units/layer.  wtt is eliminated (XT/YT crosses use
    Bt*wtx / Bt*wty).  Ops ordered so products gated only on (f1,
    qcat) run while ACT finishes the acat copies; vall second-last.
  - PE: 24 accumulating matmuls; scalar factors (-2,-4,-6...) folded
    into pre-scaled lhsT weight variants.
Tail supertile runs at 342 cols (2048 real points packed col-major
across the 6 groups) instead of a padded 512 -- ~3% less work.
l1_block is emitted two supertiles ahead of body(): its serial
mm->tanh->DVE chain fills the DVE holes at supertile boundaries
(l1 tiles have dedicated pool names to survive the longer lifetime).
xyz is preloaded to SBUF in one DMA; constants ship as two blobs
ordered so the first supertile's weights arrive first.
No GpSimd compute: Q7 tensor_tensor measures ~1.9us/512-col unit and
its queue latency stalls the strict-FIFO DVE queue (tested, slower).
Keep all tensor_tensor APs 3D ([p, k, m]); 4D broadcast ops split
into multiple instructions and schedule worse (tested, slower).
"""

import sys
from contextlib import ExitStack

import numpy as np

for _p in ("/opt/trn_rl_repo",):
    if _p not in sys.path:
        sys.path.insert(0, _p)

N_POINTS = 262144
N_CORES = 8
PPC = N_POINTS // N_CORES  # 32768
WIDTH = 20
N_HID = 7
G = 6
NPT = 512
SUPER = G * NPT            # 3072
NS = -(-PPC // SUPER)      # 11
FULL_STS = PPC // SUPER    # 10 full supertiles
REM = PPC - FULL_STS * SUPER   # 2048 points in the tail supertile
NL = (-(-REM // G) + 1) // 2 * 2   # 342 cols (even) in the tail
KDIM = G * WIDTH           # 120
MPAD = 128                 # lhsT free dim padded for FWL

# channel ids
(VAL, CH_X, CH_Y, CH_T, CH_XX, CH_XY, CH_YY, CH_XT, CH_YT,
 CH_GX, CH_GY) = range(11)

# layer-1 ctile columns: (channel, which base: 0=f1,1=f2,2=f3)
L1_COLS = [(CH_X, 0), (CH_Y, 0), (CH_T, 0),
           (CH_XX, 1), (CH_XY, 1), (CH_YY, 1), (CH_XT, 1), (CH_YT, 1),
           (CH_GX, 2), (CH_GY, 2)]

# hidden weight variants: scale per slot
W_SCALES = [1.0, -2.0, -4.0, -6.0]
S_W, S_2W, S_4W, S_6W = 0, 1, 2, 3

# layers whose f3-path squares run on ACT (DVE/ACT load balancing)
ACT_SQ_LAYERS = ()


def build_host_consts(W_in, b_in, W_hid, b_hid, W_out, b_out, lb, ub,
                      lambda_1, lambda_2):
    f32, f16 = np.float32, np.float16
    W_in = np.asarray(W_in, f32)
    b_in = np.asarray(b_in, f32)
    W_hid = np.asarray(W_hid, f32)
    b_hid = np.asarray(b_hid, f32)
    W_out = np.asarray(W_out, f32)
    b_out = np.asarray(b_out, f32)
    lb = np.asarray(lb, f32)
    ub = np.asarray(ub, f32)

    # fold normalization into layer 1
    s = (1.0 / (ub - lb)).astype(f32)
    c0 = (-lb * s).astype(f32)
    Wz = (W_in * s[:, None]).astype(f32)           # [3, 20]
    bz = (c0 @ W_in + b_in).astype(f32)            # [20]

    l1_lhsT = np.zeros((3 * G, MPAD), f32)
    hid_lhsT = np.zeros((N_HID, 4, KDIM, MPAD), f32)
    for g in range(G):
        l1_lhsT[3 * g:3 * g + 3, WIDTH * g:WIDTH * (g + 1)] = Wz
        for l in range(N_HID):
            for si, sc in enumerate(W_SCALES):
                hid_lhsT[l, si, WIDTH * g:WIDTH * (g + 1),
                         WIDTH * g:WIDTH * (g + 1)] = sc * W_hid[l]

    bias_tile = np.zeros((MPAD, 8), f32)
    bias_tile[:KDIM, 0] = np.tile(bz, G)
    for l in range(N_HID):
        bias_tile[:KDIM, 1 + l] = np.tile(b_hid[l], G)

    # layer-1 constants
    cx, cy, ct = Wz[0], Wz[1], Wz[2]
    cvecs = [cx, cy, ct,
             cx * cx, cx * cy, cy * cy, cx * ct, cy * ct,
             cx * (cx * cx + cy * cy), cy * (cx * cx + cy * cy)]
    c_tile = np.zeros((MPAD, 10), f32)
    c_tile[:KDIM] = np.stack([np.tile(v, G) for v in cvecs], axis=1)
    # layer-1 constants folded into h1 lhsT: lhsT_ch = diag(tile(c)) @ W1
    h1c = np.stack([np.tile(v, G)[:, None] * hid_lhsT[0, 0]
                    for v in cvecs])                    # [10, KDIM, MPAD]

    l1v = float(np.asarray(lambda_1).reshape(-1)[0])
    l2v = float(np.asarray(lambda_2).reshape(-1)[0])
    wpsi, wp = W_out[:, 0], W_out[:, 1]

    # output-stage lhsT bases per channel (rows at 0/32/64/96 quadrants)
    # po1: 0:12 [u,u] | 32:44 [v,v] | 64:76 [psi_xy, psi_xx]
    #      | 96:108 [psi_yy, psi_xy]
    # po2: 0:6 u | 32:38 v | 64:70 p | 96:108 [fu_lin, fv_lin]
    # fu_lin = psi_yt + p_x - l2*GY ; fv_lin = -psi_xt + p_y + l2*GX
    def base(entries):
        a = np.zeros((KDIM, MPAD), f32)
        for row0, vec in entries:
            for g in range(G):
                a[WIDTH * g:WIDTH * (g + 1), row0 + g] += vec
        return a

    o1b = {
        CH_X: base([(32, -wpsi), (38, -wpsi)]),
        CH_Y: base([(0, wpsi), (6, wpsi)]),
        CH_XX: base([(70, wpsi)]),
        CH_XY: base([(64, wpsi), (102, wpsi)]),
        CH_YY: base([(96, wpsi)]),
    }
    o2b = {
        VAL: base([(64, wp)]),
        CH_X: base([(32, -wpsi), (96, wp)]),
        CH_Y: base([(0, wpsi), (102, wp)]),
        CH_XT: base([(102, -wpsi)]),
        CH_YT: base([(96, wpsi)]),
        CH_GX: base([(102, l2v * wpsi)]),
        CH_GY: base([(96, -l2v * wpsi)]),
    }
    # piece-level lhsT lists (order must match build_program)
    o1_list = [o1b[CH_X], o1b[CH_Y],
               o1b[CH_XX], -2 * o1b[CH_XX],
               o1b[CH_XY], -2 * o1b[CH_XY],
               o1b[CH_YY], -2 * o1b[CH_YY]]
    o2_list = [o2b[VAL], o2b[CH_X], o2b[CH_Y],
               o2b[CH_XT], -2 * o2b[CH_XT],
               o2b[CH_YT], -2 * o2b[CH_YT],
               o2b[CH_GX], -6 * o2b[CH_GX], -4 * o2b[CH_GX],
               -2 * o2b[CH_GX],
               o2b[CH_GY], -2 * o2b[CH_GY], -4 * o2b[CH_GY],
               -6 * o2b[CH_GY]]

    lam_vec = np.zeros((12, 1), f32)
    lam_vec[0:6, 0] = l1v
    lam_vec[6:12, 0] = -l1v

    # batched weight blobs: wA = what the first supertile needs first
    # (l1 input weights + layer-0 variants + h1c), wB = the rest.
    wA = np.zeros((KDIM, 15 * MPAD), f32)
    wA[0:3 * G, 0:MPAD] = l1_lhsT
    for si in range(4):
        wA[:, (1 + si) * MPAD:(2 + si) * MPAD] = hid_lhsT[0, si]
    for i in range(10):
        wA[:, (5 + i) * MPAD:(6 + i) * MPAD] = h1c[i]
    oc = np.stack(o1_list + o2_list).transpose(1, 0, 2).reshape(
        KDIM, 23 * MPAD)
    wB = np.zeros((KDIM, 47 * MPAD), f32)
    for l in range(1, N_HID):
        for si in range(4):
            wB[:, ((l - 1) * 4 + si) * MPAD:((l - 1) * 4 + si + 1) * MPAD] \
                = hid_lhsT[l, si]
    wB[:, 24 * MPAD:47 * MPAD] = oc
    cf = np.zeros((MPAD, 9), f32)
    cf[:, 0:8] = bias_tile
    cf[0:12, 8] = lam_vec[:, 0]
    return dict(wA=wA.astype(f16), wB=wB.astype(f16), cf=cf,
                p_bias=float(b_out[1]))


def build_program(p_bias, ns=NS):
    import concourse.bacc as bacc
    import concourse.bass as bass
    import concourse.tile as tile
    from concourse import mybir

    f32 = mybir.dt.float32
    f16 = mybir.dt.float16
    AF = mybir.ActivationFunctionType
    OP = mybir.AluOpType

    nc = bacc.Bacc("TRN2", target_bir_lowering=False, debug=False)

    xyz_d = nc.dram_tensor("xyz", [3 * G, ns * NPT], f16,
                           kind="ExternalInput")
    wA_d = nc.dram_tensor("wA", [KDIM, 15 * MPAD], f16,
                          kind="ExternalInput")
    wB_d = nc.dram_tensor("wB", [KDIM, 47 * MPAD], f16,
                          kind="ExternalInput")
    cf_d = nc.dram_tensor("cf", [MPAD, 9], f32, kind="ExternalInput")
    u_d = nc.dram_tensor("u_out", [ns, G, NPT], f32, kind="ExternalOutput")
    v_d = nc.dram_tensor("v_out", [ns, G, NPT], f32, kind="ExternalOutput")
    p_d = nc.dram_tensor("p_out", [ns, G, NPT], f32, kind="ExternalOutput")
    fu_d = nc.dram_tensor("fu_out", [ns, G, NPT], f32, kind="ExternalOutput")
    fv_d = nc.dram_tensor("fv_out", [ns, G, NPT], f32, kind="ExternalOutput")

    with tile.TileContext(nc) as tc, ExitStack() as ctx:
        dma = nc.sync.dma_start
        act = nc.scalar.activation
        tt = nc.vector.tensor_tensor
        gtt = nc.gpsimd.tensor_tensor
        ts = nc.vector.tensor_scalar
        stt = nc.vector.scalar_tensor_tensor
        mm = nc.tensor.matmul

        # ---- persistent weights ----
        wpool = ctx.enter_context(tc.tile_pool(name="wpool", bufs=1))
        wa = wpool.tile([KDIM, 15 * MPAD], f16, name="wa")
        dma(wa[:], wA_d[:])
        cft = wpool.tile([MPAD, 9], f32, name="cf")
        dma(cft[:], cf_d[:])
        xyz_sb = wpool.tile([3 * G, NS * NPT], f16, name="xyz_sb")
        dma(xyz_sb[:], xyz_d[:])
        wb = wpool.tile([KDIM, 47 * MPAD], f16, name="wb")
        l1w = wa[0:3 * G, 0:MPAD]
        hws = [[wa[:, (1 + si) * MPAD:(2 + si) * MPAD] for si in range(4)]
               + [wa[:, (5 + i) * MPAD:(6 + i) * MPAD] for i in range(10)]]
        hws += [[wb[:, ((l - 1) * 4 + si) * MPAD:((l - 1) * 4 + si + 1)
                     * MPAD] for si in range(4)]
                for l in range(1, N_HID)]
        o1w = [wb[:, (24 + i) * MPAD:(25 + i) * MPAD] for i in range(8)]
        o2w = [wb[:, (32 + i) * MPAD:(33 + i) * MPAD] for i in range(15)]
        biases = cft[:, 0:8]
        lam = cft[0:12, 8:9]

        # ---- work pools ----
        wk = ctx.enter_context(tc.tile_pool(name="wk", bufs=3))
        wk1 = ctx.enter_context(tc.tile_pool(name="wk1", bufs=3))
        o12 = ctx.enter_context(tc.tile_pool(name="o12", bufs=2))
        o6 = ctx.enter_context(tc.tile_pool(name="o6", bufs=1))
        psT = ctx.enter_context(
            tc.tile_pool(name="psT", bufs=2, space=bass.MemorySpace.PSUM))
        psS = ctx.enter_context(
            tc.tile_pool(name="psS", bufs=2, space=bass.MemorySpace.PSUM))

        def view(ap, k, n=NPT):
            """[P, k*NPT] -> [P, k, n] (channel stride NPT, width n)"""
            v = ap.rearrange("p (k m) -> p k m", k=k)
            return v if n == NPT else v[:, :, 0:n]

        def mm_group(dst, W, pieces):
            for i, (ap, slot) in enumerate(pieces):
                mm(dst, W[slot], ap,
                   start=(i == 0), stop=(i == len(pieces) - 1))

        def hidden_layer(l, P, n):
            """P: dict ch -> list[(ap, slot)] of piece rhs APs + weight slot.
            Returns same for next layer.  n = valid cols this supertile.
            act_sq layers move the f3-path squares to ACT to balance
            the DVE/ACT load."""
            W = hws[l]
            act_sq = l in ACT_SQ_LAYERS

            tri_a = psT.tile([MPAD, 3 * NPT], f32, name="tri")
            for s, ch in enumerate([VAL, CH_X, CH_Y]):
                mm_group(tri_a[:, s * NPT:s * NPT + n], W, P[ch])
            tri_b = psT.tile([MPAD, 3 * NPT], f32, name="tri")
            for s, ch in enumerate([CH_T, CH_XX, CH_XY]):
                mm_group(tri_b[:, s * NPT:s * NPT + n], W, P[ch])
            tri_c = psT.tile([MPAD, 3 * NPT], f32, name="tri")
            for s, ch in enumerate([CH_YY, CH_XT, CH_YT]):
                mm_group(tri_c[:, s * NPT:s * NPT + n], W, P[ch])
            gx_ps = psS.tile([MPAD, NPT], f32, name="sg")
            mm_group(gx_ps[:, 0:n], W, P[CH_GX])
            gy_ps = psS.tile([MPAD, NPT], f32, name="sg")
            mm_group(gy_ps[:, 0:n], W, P[CH_GY])

            # ---- ACT staging ----
            ft = wk.tile([MPAD, 2 * NPT], f16, name="ft")   # [f1 | t0]
            act(ft[:, NPT:NPT + n], tri_a[:, 0:n], AF.Tanh,
                bias=biases[:, 1 + l:2 + l])
            t0 = ft[:, NPT:NPT + n]
            p2 = wk.tile([MPAD, NPT], f16, name="p2")
            act(p2[:, 0:n], t0, AF.Square)
            act(ft[:, 0:n], p2[:, 0:n], AF.Copy, bias=1.0, scale=-1.0)
            qcat = wk.tile([MPAD, 3 * NPT], f16, name="qcat")
            act(view(qcat[:, 0:2 * NPT], 2, n),
                view(tri_a[:, NPT:3 * NPT], 2, n), AF.Copy,
                bias=0.0, scale=1.0)
            if act_sq:
                # sq2 = ((z_x/4)^2, (z_y/4)^2) straight from PSUM; issued
                # early so tri_a's banks free before tri_c needs them
                sq2 = wk.tile([MPAD, 2 * NPT], f16, name="sq2")
                act(view(sq2[:], 2, n), view(tri_a[:, NPT:3 * NPT], 2, n),
                    AF.Square, bias=0.0, scale=0.25)
            act(qcat[:, 2 * NPT:2 * NPT + n], tri_b[:, 0:n], AF.Copy,
                bias=0.0, scale=1.0)
            # qq is only consumed by ssum (late in the DVE queue), so it
            # runs after the qcat copies to get bw started sooner
            qq = wk.tile([MPAD, NPT], f16, name="qq")
            if act_sq:
                # qq16 = 16*(6 t0^2 - 2) compensates the /16 in sq2
                act(qq[:, 0:n], p2[:, 0:n], AF.Copy, bias=-32.0, scale=96.0)
            else:
                act(qq[:, 0:n], p2[:, 0:n], AF.Copy, bias=-2.0, scale=6.0)
            acat = wk.tile([MPAD, 7 * NPT], f16, name="acat")
            act(view(acat[:, 0:2 * NPT], 2, n),
                view(tri_b[:, NPT:3 * NPT], 2, n), AF.Copy,
                bias=0.0, scale=1.0)
            act(view(acat[:, 2 * NPT:5 * NPT], 3, n),
                view(tri_c[:], 3, n), AF.Copy,
                bias=0.0, scale=1.0)
            act(acat[:, 5 * NPT:5 * NPT + n], gx_ps[:, 0:n], AF.Copy,
                bias=0.0, scale=1.0)
            act(acat[:, 6 * NPT:6 * NPT + n], gy_ps[:, 0:n], AF.Copy,
                bias=0.0, scale=1.0)

            # ---- DVE ----
            # Ops ordered so everything gated only on (p2, qcat) runs
            # while ACT finishes the acat copies; vall last.
            f1 = ft[:, 0:n]
            # bw = [Bx By Bt | wtx wty]; wtt is never needed: the
            # XT/YT crosses use Bt*wtx / Bt*wty instead of Bx/By*wtt.
            bw = wk.tile([MPAD, 5 * NPT], f16, name="bw")
            tt(view(bw[:, 0:3 * NPT], 3, n),
               f1.unsqueeze(1).broadcast_to([MPAD, 3, n]),
               view(qcat[:], 3, n), OP.mult)
            tt(view(bw[:, 3 * NPT:5 * NPT], 2, n),
               t0.unsqueeze(1).broadcast_to([MPAD, 2, n]),
               view(qcat[:, 0:2 * NPT], 2, n), OP.mult)
            Bx, By = bw[:, 0:n], bw[:, NPT:NPT + n]
            Bt = bw[:, 2 * NPT:2 * NPT + n]
            # cr1 = Bx * [wtx wty] -> (XX XY)
            cr1 = wk.tile([MPAD, 2 * NPT], f16, name="cr1")
            tt(view(cr1[:], 2, n),
               Bx.unsqueeze(1).broadcast_to([MPAD, 2, n]),
               view(bw[:, 3 * NPT:5 * NPT], 2, n), OP.mult)
            # crt = Bt * [wtx wty] -> (XT YT)
            crt = wk.tile([MPAD, 2 * NPT], f16, name="crt")
            tt(view(crt[:], 2, n),
               Bt.unsqueeze(1).broadcast_to([MPAD, 2, n]),
               view(bw[:, 3 * NPT:5 * NPT], 2, n), OP.mult)
            # cr2 = By * wty -> (YY)
            cr2 = wk.tile([MPAD, NPT], f16, name="cr2")
            tt(cr2[:, 0:n], By, bw[:, 4 * NPT:4 * NPT + n], OP.mult)
            rcat = wk.tile([MPAD, 2 * NPT], f16, name="rcat")
            if act_sq:
                # st = (zx^2+zy^2)/16 ; et = qq16*st ; rcat = et*(Bx,By)
                st = wk.tile([MPAD, NPT], f16, name="st")
                tt(st[:, 0:n], sq2[:, 0:n], sq2[:, NPT:NPT + n], OP.add)
                et = wk.tile([MPAD, NPT], f16, name="et")
                tt(et[:, 0:n], qq[:, 0:n], st[:, 0:n], OP.mult)
                tt(view(rcat[:], 2, n),
                   et[:, 0:n].unsqueeze(1).broadcast_to([MPAD, 2, n]),
                   view(bw[:, 0:2 * NPT], 2, n), OP.mult)
            else:
                # pcat = (Bx,By) * (qx,qy)
                pcat = wk.tile([MPAD, 2 * NPT], f16, name="pcat")
                tt(view(pcat[:], 2, n), view(bw[:, 0:2 * NPT], 2, n),
                   view(qcat[:, 0:2 * NPT], 2, n), OP.mult)
                psum = wk.tile([MPAD, NPT], f16, name="psum")
                tt(psum[:, 0:n], pcat[:, 0:n], pcat[:, NPT:NPT + n], OP.add)
                ssum = wk.tile([MPAD, NPT], f16, name="ssum")
                tt(ssum[:, 0:n], qq[:, 0:n], psum[:, 0:n], OP.mult)
                tt(view(rcat[:], 2, n),
                   ssum[:, 0:n].unsqueeze(1).broadcast_to([MPAD, 2, n]),
                   view(qcat[:, 0:2 * NPT], 2, n), OP.mult)
            # vall = f1 * acat (7 channels), split so the first 5 can
            # issue before the gx/gy copies land
            vall = wk.tile([MPAD, 7 * NPT], f16, name="vall")
            tt(view(vall[:], 7, n),
               f1.unsqueeze(1).broadcast_to([MPAD, 7, n]),
               view(acat[:], 7, n), OP.mult)
            # gcat[a,b] = v_a * wt_b, a in (xx,xy,yy), b in (x,y)
            gcat = wk.tile([MPAD, 6 * NPT], f16, name="gcat")
            gdst = gcat[:].rearrange("p (a b m) -> p a b m", a=3, b=2)
            if n != NPT:
                gdst = gdst[:, :, :, 0:n]
            tt(gdst,
               view(vall[:, 0:3 * NPT], 3, n).unsqueeze(2)
               .broadcast_to([MPAD, 3, 2, n]),
               view(bw[:, 3 * NPT:5 * NPT], 2, n).unsqueeze(1)
               .broadcast_to([MPAD, 3, 2, n]),
               OP.mult)

            def sl(tile_, k):
                return tile_[0:KDIM, k * NPT:k * NPT + n]

            return {
                VAL: [(ft[0:KDIM, NPT:NPT + n], S_W)],
                CH_X: [(sl(bw, 0), S_W)],
                CH_Y: [(sl(bw, 1), S_W)],
                CH_T: [(sl(bw, 2), S_W)],
                CH_XX: [(sl(vall, 0), S_W), (sl(cr1, 0), S_2W)],
                CH_XY: [(sl(vall, 1), S_W), (sl(cr1, 1), S_2W)],
                CH_YY: [(sl(vall, 2), S_W), (sl(cr2, 0), S_2W)],
                CH_XT: [(sl(vall, 3), S_W), (sl(crt, 0), S_2W)],
                CH_YT: [(sl(vall, 4), S_W), (sl(crt, 1), S_2W)],
                CH_GX: [(sl(vall, 5), S_W), (sl(rcat, 0), S_W),
                        (sl(gcat, 0), S_6W), (sl(gcat, 3), S_4W),
                        (sl(gcat, 4), S_2W)],
                CH_GY: [(sl(vall, 6), S_W), (sl(rcat, 1), S_W),
                        (sl(gcat, 1), S_2W), (sl(gcat, 2), S_4W),
                        (sl(gcat, 5), S_6W)],
            }

        def l1_block(sidx, n):
            xt = xyz_sb[:, sidx * NPT:sidx * NPT + n]
            ps0 = psS.tile([MPAD, NPT], f32, name="sg")
            mm(ps0[:, 0:n], l1w, xt, start=True, stop=True)
            ft1 = wk.tile([MPAD, 2 * NPT], f16, name="ftl1")
            act(ft1[:, NPT:NPT + n], ps0[:, 0:n], AF.Tanh,
                bias=biases[:, 0:1])
            t01 = ft1[:, NPT:NPT + n]
            p21 = wk.tile([MPAD, NPT], f16, name="p2l1")
            tt(p21[:, 0:n], t01, t01, OP.mult)
            f11 = ft1[:, 0:n]
            ts(f11, p21[:, 0:n], -1.0, 1.0, OP.mult, OP.add)
            qq1 = wk.tile([MPAD, NPT], f16, name="qql1")
            ts(qq1[:, 0:n], p21[:, 0:n], 6.0, -2.0, OP.mult, OP.add)
            mneg = wk1.tile([MPAD, NPT], f16, name="mneg")
            ts(mneg[:, 0:n], t01, -2.0, None, OP.mult)
            ff2 = wk1.tile([MPAD, NPT], f16, name="ff2")
            tt(ff2[:, 0:n], mneg[:, 0:n], f11, OP.mult)
            ff3 = wk1.tile([MPAD, NPT], f16, name="ff3")
            tt(ff3[:, 0:n], qq1[:, 0:n], f11, OP.mult)
            srcs = {0: ft1[0:KDIM, 0:n], 1: ff2[0:KDIM, 0:n],
                    2: ff3[0:KDIM, 0:n]}
            P = {VAL: [(ft1[0:KDIM, NPT:NPT + n], S_W)]}
            for k, (ch, b) in enumerate(L1_COLS):
                P[ch] = [(srcs[b], 4 + k)]
            return P

        def body(sidx, P, n):
            # ---- hidden layers ----
            for l in range(N_HID):
                P = hidden_layer(l, P, n)

            # ---- output stage ----
            po1t = psS.tile([MPAD, NPT], f32, name="sg")
            po2t = psS.tile([MPAD, NPT], f32, name="sg")
            po1 = po1t[:, 0:n]
            po2 = po2t[:, 0:n]
            p1_srcs = [P[CH_X][0], P[CH_Y][0],
                       P[CH_XX][0], P[CH_XX][1],
                       P[CH_XY][0], P[CH_XY][1],
                       P[CH_YY][0], P[CH_YY][1]]
            for i, (ap, _) in enumerate(p1_srcs):
                mm(po1, o1w[i], ap, start=(i == 0),
                   stop=(i == len(p1_srcs) - 1))
            # (piece, o2w index); GX/GY v- and r-pieces share base weights
            p2_srcs = [(P[VAL][0], 0), (P[CH_X][0], 1), (P[CH_Y][0], 2),
                       (P[CH_XT][0], 3), (P[CH_XT][1], 4),
                       (P[CH_YT][0], 5), (P[CH_YT][1], 6),
                       (P[CH_GX][0], 7), (P[CH_GX][1], 7),
                       (P[CH_GX][2], 8), (P[CH_GX][3], 9), (P[CH_GX][4], 10),
                       (P[CH_GY][0], 11), (P[CH_GY][1], 11),
                       (P[CH_GY][2], 12), (P[CH_GY][3], 13),
                       (P[CH_GY][4], 14)]
            for i, ((ap, _), wi) in enumerate(p2_srcs):
                mm(po2, o2w[wi], ap, start=(i == 0),
                   stop=(i == len(p2_srcs) - 1))

            a1 = o12.tile([12, NPT], f32, name="a1")
            act(a1[:, 0:n], po1t[0:12, 0:n], AF.Copy, bias=0.0, scale=1.0)
            a2 = o12.tile([12, NPT], f32, name="a2")
            act(a2[:, 0:n], po1t[32:44, 0:n], AF.Copy, bias=0.0, scale=1.0)
            pl1 = o12.tile([12, NPT], f32, name="pl1")
            tt(pl1[:, 0:n], a1[:, 0:n], po1t[64:76, 0:n], OP.mult)
            pl2 = o12.tile([12, NPT], f32, name="pl2")
            tt(pl2[:, 0:n], a2[:, 0:n], po1t[96:108, 0:n], OP.mult)
            dd = o12.tile([12, NPT], f32, name="dd")
            tt(dd[:, 0:n], pl1[:, 0:n], pl2[:, 0:n], OP.add)
            ff = o12.tile([12, NPT], f32, name="ff")
            stt(ff[:, 0:n], dd[:, 0:n], lam, po2t[96:108, 0:n],
                OP.mult, OP.add)
            pp = o6.tile([6, NPT], f32, name="pp")
            act(pp[:, 0:n], po2t[64:70, 0:n], AF.Copy,
                bias=float(p_bias), scale=1.0)
            uu = o6.tile([6, NPT], f32, name="uu")
            act(uu[:, 0:n], po2t[0:6, 0:n], AF.Copy, bias=0.0, scale=1.0)
            vv = o6.tile([6, NPT], f32, name="vv")
            act(vv[:, 0:n], po2t[32:38, 0:n], AF.Copy, bias=0.0, scale=1.0)

            dma(u_d[sidx][:, 0:n], uu[0:6, 0:n])
            dma(v_d[sidx][:, 0:n], vv[0:6, 0:n])
            dma(p_d[sidx][:, 0:n], pp[0:6, 0:n])
            dma(fu_d[sidx][:, 0:n], ff[0:6, 0:n])
            dma(fv_d[sidx][:, 0:n], ff[6:12, 0:n])

        def n_of(s):
            return NPT if s < FULL_STS else NL

        # l1 runs two supertiles ahead: its serial mm->tanh->DVE chain
        # fills the DVE holes at supertile boundaries
        Pq = {0: l1_block(0, n_of(0))}
        dma(wb[:], wB_d[:])
        if ns > 1:
            Pq[1] = l1_block(1, n_of(1))
        for s in range(ns):
            if s + 2 < ns:
                Pq[s + 2] = l1_block(s + 2, n_of(s + 2))
            body(s, Pq.pop(s), n_of(s))

    nc.compile()
    return nc


def make_in_maps(inputs, consts, ns=NS):
    x = np.asarray(inputs["x"], np.float32).reshape(-1)
    y = np.asarray(inputs["y"], np.float32).reshape(-1)
    t = np.asarray(inputs["t"], np.float32).reshape(-1)
    padpc = ns * SUPER
    shared = {k: consts[k] for k in ("wA", "wB", "cf")}
    in_maps = []
    for c in range(N_CORES):
        sl = slice(c * PPC, (c + 1) * PPC)

        def lay(vec):
            seg = vec[sl]
            out = np.zeros((ns, G, NPT), np.float32)
            out[:FULL_STS] = seg[:FULL_STS * SUPER].reshape(
                FULL_STS, G, NPT)
            last = np.zeros((G * NL,), np.float32)
            last[:REM] = seg[FULL_STS * SUPER:]
            out[FULL_STS, :, :NL] = last.reshape(G, NL)
            return out

        xyz = np.zeros((ns, 3 * G, NPT), np.float32)
        xyz[:, 0::3, :] = lay(x)
        xyz[:, 1::3, :] = lay(y)
        xyz[:, 2::3, :] = lay(t)
        xyz = np.ascontiguousarray(
            xyz.transpose(1, 0, 2).reshape(3 * G, ns * NPT))
        in_maps.append({"xyz": xyz.astype(np.float16), **shared})
    return in_maps


def unshard_core(a):
    """[NS, G, NPT] per-core output -> flat [PPC] in point order."""
    a = np.asarray(a).reshape(NS, G, NPT)
    head = a[:FULL_STS].reshape(-1)
    tail = a[FULL_STS, :, :NL].reshape(-1)[:REM]
    return np.concatenate([head, tail])


def kernel(**inputs):
    consts = build_host_consts(
        inputs["W_in"], inputs["b_in"], inputs["W_hid"], inputs["b_hid"],
        inputs["W_out"], inputs["b_out"], inputs["lb"], inputs["ub"],
        inputs["lambda_1"], inputs["lambda_2"])
    nc = build_program(consts["p_bias"])
    in_maps = make_in_maps(inputs, consts)

    from concourse.bass_utils import run_bass_kernel_spmd
    res = run_bass_kernel_spmd(nc, in_maps, list(range(N_CORES)))

    outs = []
    for name in ("u_out", "v_out", "p_out", "fu_out", "fv_out"):
        full = np.concatenate(
            [unshard_core(res.results[c][name])
             for c in range(N_CORES)])
        outs.append(np.ascontiguousarray(full[:, None], dtype=np.float32))
    return tuple(outs)



# revision 44
# speedup vs baseline: 1.0428x; 1.0428x over previous
"""Trainium2 Bass kernel v4 for PhysicsInformedNN (Navier-Stokes PINN).

Jet propagation with 11 channels: val, x, y, t, xx, xy, yy, xt, yt,
gx (= psi_xxx + psi_xyy), gy (= psi_xxy + psi_yyy).  The Laplacian
contraction works because third derivatives only enter f_u/f_v via
u_xx+u_yy = d_y(Lap psi) and v_xx+v_yy = -d_x(Lap psi).

Per hidden layer, per supertile of 6x512 points:
  - PSUM: tri tiles [128,1536] (3 banks) x bufs=2 + pair [128,1024]:
    tri_a=(VAL,X,Y) tri_b=(T,XX,XY) tri_c=(YY,XT,YT) pair=(GX,GY)
  - ACT: tanh, t0^2, f1=1-t0^2, qq + batched PSUM->SBUF fp16 copies
  - DVE: wide fp16 2x-mode tensor_tensor with stride-0 broadcast APs;
    29 product-units/layer.  wtt is eliminated (XT/YT crosses use
    Bt*wtx / Bt*wty).  Ops ordered so products gated only on (f1,
    qcat) run while ACT finishes the acat copies; vall second-last.
  - PE: 24 accumulating matmuls; scalar factors (-2,-4,-6...) folded
    into pre-scaled lhsT weight variants.
Tail supertile runs at 342 cols (2048 real points packed col-major
across the 6 groups) instead of a padded 512 -- ~3% less work.
l1_block is emitted two supertiles ahead of body(): its serial
mm->tanh->DVE chain fills the DVE holes at supertile boundaries
(l1 tiles have dedicated pool names to survive the longer lifetime).
xyz is preloaded to SBUF in one DMA; constants ship as two blobs
ordered so the first supertile's weights arrive first.
No GpSimd compute: Q7 tensor_tensor measures ~1.9us/512-col unit and
its queue latency stalls the strict-FIFO DVE queue (tested, slower).
Keep all tensor_tensor APs 3D ([p, k, m]); 4D broadcast ops split
into multiple instructions and schedule worse (tested, slower).
"""

import sys
from contextlib import ExitStack

import numpy as np

for _p in ("/opt/trn_rl_repo",):
    if _p not in sys.path:
        sys.path.insert(0, _p)

N_POINTS = 262144
N_CORES = 8
PPC = N_POINTS // N_CORES  # 32768
WIDTH = 20
N_HID = 7
G = 6
NPT = 512
SUPER = G * NPT            # 3072
NS = -(-PPC // SUPER)      # 11
FULL_STS = PPC // SUPER    # 10 full supertiles
REM = PPC - FULL_STS * SUPER   # 2048 points in the tail supertile
NL = (-(-REM // G) + 1) // 2 * 2   # 342 cols (even) in the tail
KDIM = G * WIDTH           # 120
MPAD = 128                 # lhsT free dim padded for FWL

# channel ids
(VAL, CH_X, CH_Y, CH_T, CH_XX, CH_XY, CH_YY, CH_XT, CH_YT,
 CH_GX, CH_GY) = range(11)

# layer-1 ctile columns: (channel, which base: 0=f1,1=f2,2=f3)
L1_COLS = [(CH_X, 0), (CH_Y, 0), (CH_T, 0),
           (CH_XX, 1), (CH_XY, 1), (CH_YY, 1), (CH_XT, 1), (CH_YT, 1),
           (CH_GX, 2), (CH_GY, 2)]

# hidden weight variants: scale per slot
W_SCALES = [1.0, -2.0, -4.0, -6.0]
S_W, S_2W, S_4W, S_6W = 0, 1, 2, 3

# layers whose f3-path squares run on ACT (DVE/ACT load balancing)
ACT_SQ_LAYERS = ()


def build_host_consts(W_in, b_in, W_hid, b_hid, W_out, b_out, lb, ub,
                      lambda_1, lambda_2):
    f32, f16 = np.float32, np.float16
    W_in = np.asarray(W_in, f32)
    b_in = np.asarray(b_in, f32)
    W_hid = np.asarray(W_hid, f32)
    b_hid = np.asarray(b_hid, f32)
    W_out = np.asarray(W_out, f32)
    b_out = np.asarray(b_out, f32)
    lb = np.asarray(lb, f32)
    ub = np.asarray(ub, f32)

    # fold normalization into layer 1
    s = (1.0 / (ub - lb)).astype(f32)
    c0 = (-lb * s).astype(f32)
    Wz = (W_in * s[:, None]).astype(f32)           # [3, 20]
    bz = (c0 @ W_in + b_in).astype(f32)            # [20]

    l1_lhsT = np.zeros((3 * G, MPAD), f32)
    hid_lhsT = np.zeros((N_HID, 4, KDIM, MPAD), f32)
    for g in range(G):
        l1_lhsT[3 * g:3 * g + 3, WIDTH * g:WIDTH * (g + 1)] = Wz
        for l in range(N_HID):
            for si, sc in enumerate(W_SCALES):
                hid_lhsT[l, si, WIDTH * g:WIDTH * (g + 1),
                         WIDTH * g:WIDTH * (g + 1)] = sc * W_hid[l]

    bias_tile = np.zeros((MPAD, 8), f32)
    bias_tile[:KDIM, 0] = np.tile(bz, G)
    for l in range(N_HID):
        bias_tile[:KDIM, 1 + l] = np.tile(b_hid[l], G)

    # layer-1 constants
    cx, cy, ct = Wz[0], Wz[1], Wz[2]
    cvecs = [cx, cy, ct,
             cx * cx, cx * cy, cy * cy, cx * ct, cy * ct,
             cx * (cx * cx + cy * cy), cy * (cx * cx + cy * cy)]
    c_tile = np.zeros((MPAD, 10), f32)
    c_tile[:KDIM] = np.stack([np.tile(v, G) for v in cvecs], axis=1)
    # layer-1 constants folded into h1 lhsT: lhsT_ch = diag(tile(c)) @ W1
    h1c = np.stack([np.tile(v, G)[:, None] * hid_lhsT[0, 0]
                    for v in cvecs])                    # [10, KDIM, MPAD]

    l1v = float(np.asarray(lambda_1).reshape(-1)[0])
    l2v = float(np.asarray(lambda_2).reshape(-1)[0])
    wpsi, wp = W_out[:, 0], W_out[:, 1]

    # output-stage lhsT bases per channel (rows at 0/32/64/96 quadrants)
    # po1: 0:12 [u,u] | 32:44 [v,v] | 64:76 [psi_xy, psi_xx]
    #      | 96:108 [psi_yy, psi_xy]
    # po2: 0:6 u | 32:38 v | 64:70 p | 96:108 [fu_lin, fv_lin]
    # fu_lin = psi_yt + p_x - l2*GY ; fv_lin = -psi_xt + p_y + l2*GX
    def base(entries):
        a = np.zeros((KDIM, MPAD), f32)
        for row0, vec in entries:
            for g in range(G):
                a[WIDTH * g:WIDTH * (g + 1), row0 + g] += vec
        return a

    o1b = {
        CH_X: base([(32, -wpsi), (38, -wpsi)]),
        CH_Y: base([(0, wpsi), (6, wpsi)]),
        CH_XX: base([(70, wpsi)]),
        CH_XY: base([(64, wpsi), (102, wpsi)]),
        CH_YY: base([(96, wpsi)]),
    }
    o2b = {
        VAL: base([(64, wp)]),
        CH_X: base([(32, -wpsi), (96, wp)]),
        CH_Y: base([(0, wpsi), (102, wp)]),
        CH_XT: base([(102, -wpsi)]),
        CH_YT: base([(96, wpsi)]),
        CH_GX: base([(102, l2v * wpsi)]),
        CH_GY: base([(96, -l2v * wpsi)]),
    }
    # piece-level lhsT lists (order must match build_program)
    o1_list = [o1b[CH_X], o1b[CH_Y],
               o1b[CH_XX], -2 * o1b[CH_XX],
               o1b[CH_XY], -2 * o1b[CH_XY],
               o1b[CH_YY], -2 * o1b[CH_YY]]
    o2_list = [o2b[VAL], o2b[CH_X], o2b[CH_Y],
               o2b[CH_XT], -2 * o2b[CH_XT],
               o2b[CH_YT], -2 * o2b[CH_YT],
               o2b[CH_GX], -6 * o2b[CH_GX], -4 * o2b[CH_GX],
               -2 * o2b[CH_GX],
               o2b[CH_GY], -2 * o2b[CH_GY], -4 * o2b[CH_GY],
               -6 * o2b[CH_GY]]

    lam_vec = np.zeros((12, 1), f32)
    lam_vec[0:6, 0] = l1v
    lam_vec[6:12, 0] = -l1v

    # batched weight blobs: wA = what the first supertile needs first
    # (l1 input weights + layer-0 variants + h1c), wB = the rest.
    wA = np.zeros((KDIM, 15 * MPAD), f32)
    wA[0:3 * G, 0:MPAD] = l1_lhsT
    for si in range(4):
        wA[:, (1 + si) * MPAD:(2 + si) * MPAD] = hid_lhsT[0, si]
    for i in range(10):
        wA[:, (5 + i) * MPAD:(6 + i) * MPAD] = h1c[i]
    oc = np.stack(o1_list + o2_list).transpose(1, 0, 2).reshape(
        KDIM, 23 * MPAD)
    wB = np.zeros((KDIM, 47 * MPAD), f32)
    for l in range(1, N_HID):
        for si in range(4):
            wB[:, ((l - 1) * 4 + si) * MPAD:((l - 1) * 4 + si + 1) * MPAD] \
                = hid_lhsT[l, si]
    wB[:, 24 * MPAD:47 * MPAD] = oc
    cf = np.zeros((MPAD, 9), f32)
    cf[:, 0:8] = bias_tile
    cf[0:12, 8] = lam_vec[:, 0]
    return dict(wA=wA.astype(f16), wB=wB.astype(f16), cf=cf,
                p_bias=float(b_out[1]))


def build_program(p_bias, ns=NS):
    import concourse.bacc as bacc
    import concourse.bass as bass
    import concourse.tile as tile
    from concourse import mybir

    f32 = mybir.dt.float32
    f16 = mybir.dt.float16
    AF = mybir.ActivationFunctionType
    OP = mybir.AluOpType

    nc = bacc.Bacc("TRN2", target_bir_lowering=False, debug=False)

    xyz_d = nc.dram_tensor("xyz", [3 * G, ns * NPT], f16,
                           kind="ExternalInput")
    wA_d = nc.dram_tensor("wA", [KDIM, 15 * MPAD], f16,
                          kind="ExternalInput")
    wB_d = nc.dram_tensor("wB", [KDIM, 47 * MPAD], f16,
                          kind="ExternalInput")
    cf_d = nc.dram_tensor("cf", [MPAD, 9], f32, kind="ExternalInput")
    u_d = nc.dram_tensor("u_out", [ns, G, NPT], f32, kind="ExternalOutput")
    v_d = nc.dram_tensor("v_out", [ns, G, NPT], f32, kind="ExternalOutput")
    p_d = nc.dram_tensor("p_out", [ns, G, NPT], f32, kind="ExternalOutput")
    fu_d = nc.dram_tensor("fu_out", [ns, G, NPT], f32, kind="ExternalOutput")
    fv_d = nc.dram_tensor("fv_out", [ns, G, NPT], f32, kind="ExternalOutput")

    with tile.TileContext(nc) as tc, ExitStack() as ctx:
        dma = nc.sync.dma_start
        act = nc.scalar.activation
        tt = nc.vector.tensor_tensor
        gtt = nc.gpsimd.tensor_tensor
        ts = nc.vector.tensor_scalar
        stt = nc.vector.scalar_tensor_tensor
        mm = nc.tensor.matmul

        # ---- persistent weights ----
        wpool = ctx.enter_context(tc.tile_pool(name="wpool", bufs=1))
        wa = wpool.tile([KDIM, 15 * MPAD], f16, name="wa")
        dma(wa[:], wA_d[:])
        cft = wpool.tile([MPAD, 9], f32, name="cf")
        dma(cft[:], cf_d[:])
        xyz_sb = wpool.tile([3 * G, NS * NPT], f16, name="xyz_sb")
        dma(xyz_sb[:], xyz_d[:])
        wb = wpool.tile([KDIM, 47 * MPAD], f16, name="wb")
        l1w = wa[0:3 * G, 0:MPAD]
        hws = [[wa[:, (1 + si) * MPAD:(2 + si) * MPAD] for si in range(4)]
               + [wa[:, (5 + i) * MPAD:(6 + i) * MPAD] for i in range(10)]]
        hws += [[wb[:, ((l - 1) * 4 + si) * MPAD:((l - 1) * 4 + si + 1)
                     * MPAD] for si in range(4)]
                for l in range(1, N_HID)]
        o1w = [wb[:, (24 + i) * MPAD:(25 + i) * MPAD] for i in range(8)]
        o2w = [wb[:, (32 + i) * MPAD:(33 + i) * MPAD] for i in range(15)]
        biases = cft[:, 0:8]
        lam = cft[0:12, 8:9]

        # ---- work pools ----
        wk = ctx.enter_context(tc.tile_pool(name="wk", bufs=3))
        wk1 = ctx.enter_context(tc.tile_pool(name="wk1", bufs=3))
        o12 = ctx.enter_context(tc.tile_pool(name="o12", bufs=2))
        o6 = ctx.enter_context(tc.tile_pool(name="o6", bufs=1))
        psT = ctx.enter_context(
            tc.tile_pool(name="psT", bufs=2, space=bass.MemorySpace.PSUM))
        psS = ctx.enter_context(
            tc.tile_pool(name="psS", bufs=2, space=bass.MemorySpace.PSUM))

        def view(ap, k, n=NPT):
            """[P, k*NPT] -> [P, k, n] (channel stride NPT, width n)"""
            v = ap.rearrange("p (k m) -> p k m", k=k)
            return v if n == NPT else v[:, :, 0:n]

        def mm_group(dst, W, pieces):
            for i, (ap, slot) in enumerate(pieces):
                mm(dst, W[slot], ap,
                   start=(i == 0), stop=(i == len(pieces) - 1))

        def hidden_layer(l, P, n):
            """P: dict ch -> list[(ap, slot)] of piece rhs APs + weight slot.
            Returns same for next layer.  n = valid cols this supertile.
            act_sq layers move the f3-path squares to ACT to balance
            the DVE/ACT load."""
            W = hws[l]
            act_sq = l in ACT_SQ_LAYERS

            tri_a = psT.tile([MPAD, 3 * NPT], f32, name="tri")
            for s, ch in enumerate([VAL, CH_X, CH_Y]):
                mm_group(tri_a[:, s * NPT:s * NPT + n], W, P[ch])
            tri_b = psT.tile([MPAD, 3 * NPT], f32, name="tri")
            for s, ch in enumerate([CH_T, CH_XX, CH_XY]):
                mm_group(tri_b[:, s * NPT:s * NPT + n], W, P[ch])
            tri_c = psT.tile([MPAD, 3 * NPT], f32, name="tri")
            for s, ch in enumerate([CH_YY, CH_XT, CH_YT]):
                mm_group(tri_c[:, s * NPT:s * NPT + n], W, P[ch])
            gx_ps = psS.tile([MPAD, NPT], f32, name="sg")
            mm_group(gx_ps[:, 0:n], W, P[CH_GX])
            gy_ps = psS.tile([MPAD, NPT], f32, name="sg")
            mm_group(gy_ps[:, 0:n], W, P[CH_GY])

            # ---- ACT staging ----
            ft = wk.tile([MPAD, 2 * NPT], f16, name="ft")   # [f1 | t0]
            act(ft[:, NPT:NPT + n], tri_a[:, 0:n], AF.Tanh,
                bias=biases[:, 1 + l:2 + l])
            t0 = ft[:, NPT:NPT + n]
            p2 = wk.tile([MPAD, NPT], f16, name="p2")
            act(p2[:, 0:n], t0, AF.Square)
            act(ft[:, 0:n], p2[:, 0:n], AF.Copy, bias=1.0, scale=-1.0)
            qcat = wk.tile([MPAD, 3 * NPT], f16, name="qcat")
            act(view(qcat[:, 0:2 * NPT], 2, n),
                view(tri_a[:, NPT:3 * NPT], 2, n), AF.Copy,
                bias=0.0, scale=1.0)
            if act_sq:
                # sq2 = ((z_x/4)^2, (z_y/4)^2) straight from PSUM; issued
                # early so tri_a's banks free before tri_c needs them
                sq2 = wk.tile([MPAD, 2 * NPT], f16, name="sq2")
                act(view(sq2[:], 2, n), view(tri_a[:, NPT:3 * NPT], 2, n),
                    AF.Square, bias=0.0, scale=0.25)
            act(qcat[:, 2 * NPT:2 * NPT + n], tri_b[:, 0:n], AF.Copy,
                bias=0.0, scale=1.0)
            # qq is only consumed by ssum (late in the DVE queue), so it
            # runs after the qcat copies to get bw started sooner
            qq = wk.tile([MPAD, NPT], f16, name="qq")
            if act_sq:
                # qq16 = 16*(6 t0^2 - 2) compensates the /16 in sq2
                act(qq[:, 0:n], p2[:, 0:n], AF.Copy, bias=-32.0, scale=96.0)
            else:
                act(qq[:, 0:n], p2[:, 0:n], AF.Copy, bias=-2.0, scale=6.0)
            acat = wk.tile([MPAD, 7 * NPT], f16, name="acat")
            act(view(acat[:, 0:2 * NPT], 2, n),
                view(tri_b[:, NPT:3 * NPT], 2, n), AF.Copy,
                bias=0.0, scale=1.0)
            act(view(acat[:, 2 * NPT:5 * NPT], 3, n),
                view(tri_c[:], 3, n), AF.Copy,
                bias=0.0, scale=1.0)
            act(acat[:, 5 * NPT:5 * NPT + n], gx_ps[:, 0:n], AF.Copy,
                bias=0.0, scale=1.0)
            act(acat[:, 6 * NPT:6 * NPT + n], gy_ps[:, 0:n], AF.Copy,
                bias=0.0, scale=1.0)

            # ---- DVE ----
            # Ops ordered so everything gated only on (p2, qcat) runs
            # while ACT finishes the acat copies; vall last.
            f1 = ft[:, 0:n]
            # bw = [Bx By Bt | wtx wty]; wtt is never needed: the
            # XT/YT crosses use Bt*wtx / Bt*wty instead of Bx/By*wtt.
            bw = wk.tile([MPAD, 5 * NPT], f16, name="bw")
            tt(view(bw[:, 0:3 * NPT], 3, n),
               f1.unsqueeze(1).broadcast_to([MPAD, 3, n]),
               view(qcat[:], 3, n), OP.mult)
            tt(view(bw[:, 3 * NPT:5 * NPT], 2, n),
               t0.unsqueeze(1).broadcast_to([MPAD, 2, n]),
               view(qcat[:, 0:2 * NPT], 2, n), OP.mult)
            Bx, By = bw[:, 0:n], bw[:, NPT:NPT + n]
            Bt = bw[:, 2 * NPT:2 * NPT + n]
            # cr1 = Bx * [wtx wty] -> (XX XY)
            cr1 = wk.tile([MPAD, 2 * NPT], f16, name="cr1")
            tt(view(cr1[:], 2, n),
               Bx.unsqueeze(1).broadcast_to([MPAD, 2, n]),
               view(bw[:, 3 * NPT:5 * NPT], 2, n), OP.mult)
            # crt = Bt * [wtx wty] -> (XT YT)
            crt = wk.tile([MPAD, 2 * NPT], f16, name="crt")
            tt(view(crt[:], 2, n),
               Bt.unsqueeze(1).broadcast_to([MPAD, 2, n]),
               view(bw[:, 3 * NPT:5 * NPT], 2, n), OP.mult)
            # cr2 = By * wty -> (YY)
            cr2 = wk.tile([MPAD, NPT], f16, name="cr2")
            tt(cr2[:, 0:n], By, bw[:, 4 * NPT:4 * NPT + n], OP.mult)
            rcat = wk.tile([MPAD, 2 * NPT], f16, name="rcat")
            if act_sq:
                # st = (zx^2+zy^2)/16 ; et = qq16*st ; rcat = et*(Bx,By)
                st = wk.tile([MPAD, NPT], f16, name="st")
                tt(st[:, 0:n], sq2[:, 0:n], sq2[:, NPT:NPT + n], OP.add)
                et = wk.tile([MPAD, NPT], f16, name="et")
                tt(et[:, 0:n], qq[:, 0:n], st[:, 0:n], OP.mult)
                tt(view(rcat[:], 2, n),
                   et[:, 0:n].unsqueeze(1).broadcast_to([MPAD, 2, n]),
                   view(bw[:, 0:2 * NPT], 2, n), OP.mult)
            else:
                # pcat = (Bx,By) * (qx,qy)
                pcat = wk.tile([MPAD, 2 * NPT], f16, name="pcat")
                tt(view(pcat[:], 2, n), view(bw[:, 0:2 * NPT], 2, n),
                   view(qcat[:, 0:2 * NPT], 2, n), OP.mult)
                psum = wk.tile([MPAD, NPT], f16, name="psum")
                tt(psum[:, 0:n], pcat[:, 0:n], pcat[:, NPT:NPT + n], OP.add)
                ssum = wk.tile([MPAD, NPT], f16, name="ssum")
                tt(ssum[:, 0:n], qq[:, 0:n], psum[:, 0:n], OP.mult)
                tt(view(rcat[:], 2, n),
                   ssum[:, 0:n].unsqueeze(1).broadcast_to([MPAD, 2, n]),
                   view(qcat[:, 0:2 * NPT], 2, n), OP.mult)
            # vall = f1 * acat (7 channels), split so the first 5 can
            # issue before the gx/gy copies land
            vall = wk.tile([MPAD, 7 * NPT], f16, name="vall")
            tt(view(vall[:, 0:5 * NPT], 5, n),
               f1.unsqueeze(1).broadcast_to([MPAD, 5, n]),
               view(acat[:, 0:5 * NPT], 5, n), OP.mult)
            tt(view(vall[:, 5 * NPT:7 * NPT], 2, n),
               f1.unsqueeze(1).broadcast_to([MPAD, 2, n]),
               view(acat[:, 5 * NPT:7 * NPT], 2, n), OP.mult)
            # gcat[a,b] = v_a * wt_b, a in (xx,xy,yy), b in (x,y)
            gcat = wk.tile([MPAD, 6 * NPT], f16, name="gcat")
            gdst = gcat[:].rearrange("p (a b m) -> p a b m", a=3, b=2)
            if n != NPT:
                gdst = gdst[:, :, :, 0:n]
            tt(gdst,
               view(vall[:, 0:3 * NPT], 3, n).unsqueeze(2)
               .broadcast_to([MPAD, 3, 2, n]),
               view(bw[:, 3 * NPT:5 * NPT], 2, n).unsqueeze(1)
               .broadcast_to([MPAD, 3, 2, n]),
               OP.mult)

            def sl(tile_, k):
                return tile_[0:KDIM, k * NPT:k * NPT + n]

            return {
                VAL: [(ft[0:KDIM, NPT:NPT + n], S_W)],
                CH_X: [(sl(bw, 0), S_W)],
                CH_Y: [(sl(bw, 1), S_W)],
                CH_T: [(sl(bw, 2), S_W)],
                CH_XX: [(sl(vall, 0), S_W), (sl(cr1, 0), S_2W)],
                CH_XY: [(sl(vall, 1), S_W), (sl(cr1, 1), S_2W)],
                CH_YY: [(sl(vall, 2), S_W), (sl(cr2, 0), S_2W)],
                CH_XT: [(sl(vall, 3), S_W), (sl(crt, 0), S_2W)],
                CH_YT: [(sl(vall, 4), S_W), (sl(crt, 1), S_2W)],
                CH_GX: [(sl(vall, 5), S_W), (sl(rcat, 0), S_W),
                        (sl(gcat, 0), S_6W), (sl(gcat, 3), S_4W),
                        (sl(gcat, 4), S_2W)],
                CH_GY: [(sl(vall, 6), S_W), (sl(rcat, 1), S_W),
                        (sl(gcat, 1), S_2W), (sl(gcat, 2), S_4W),
                        (sl(gcat, 5), S_6W)],
            }

        def l1_block(sidx, n):
            xt = xyz_sb[:, sidx * NPT:sidx * NPT + n]
            ps0 = psS.tile([MPAD, NPT], f32, name="sg")
            mm(ps0[:, 0:n], l1w, xt, start=True, stop=True)
            ft1 = wk.tile([MPAD, 2 * NPT], f16, name="ftl1")
            act(ft1[:, NPT:NPT + n], ps0[:, 0:n], AF.Tanh,
                bias=biases[:, 0:1])
            t01 = ft1[:, NPT:NPT + n]
            p21 = wk.tile([MPAD, NPT], f16, name="p2l1")
            tt(p21[:, 0:n], t01, t01, OP.mult)
            f11 = ft1[:, 0:n]
            ts(f11, p21[:, 0:n], -1.0, 1.0, OP.mult, OP.add)
            qq1 = wk.tile([MPAD, NPT], f16, name="qql1")
            ts(qq1[:, 0:n], p21[:, 0:n], 6.0, -2.0, OP.mult, OP.add)
            mneg = wk1.tile([MPAD, NPT], f16, name="mneg")
            ts(mneg[:, 0:n], t01, -2.0, None, OP.mult)
            ff2 = wk1.tile([MPAD, NPT], f16, name="ff2")
            tt(ff2[:, 0:n], mneg[:, 0:n], f11, OP.mult)
            ff3 = wk1.tile([MPAD, NPT], f16, name="ff3")
            tt(ff3[:, 0:n], qq1[:, 0:n], f11, OP.mult)
            srcs = {0: ft1[0:KDIM, 0:n], 1: ff2[0:KDIM, 0:n],
                    2: ff3[0:KDIM, 0:n]}
            P = {VAL: [(ft1[0:KDIM, NPT:NPT + n], S_W)]}
            for k, (ch, b) in enumerate(L1_COLS):
                P[ch] = [(srcs[b], 4 + k)]
            return P

        def body(sidx, P, n):
            # ---- hidden layers ----
            for l in range(N_HID):
                P = hidden_layer(l, P, n)

            # ---- output stage ----
            po1t = psS.tile([MPAD, NPT], f32, name="sg")
            po2t = psS.tile([MPAD, NPT], f32, name="sg")
            po1 = po1t[:, 0:n]
            po2 = po2t[:, 0:n]
            p1_srcs = [P[CH_X][0], P[CH_Y][0],
                       P[CH_XX][0], P[CH_XX][1],
                       P[CH_XY][0], P[CH_XY][1],
                       P[CH_YY][0], P[CH_YY][1]]
            for i, (ap, _) in enumerate(p1_srcs):
                mm(po1, o1w[i], ap, start=(i == 0),
                   stop=(i == len(p1_srcs) - 1))
            # (piece, o2w index); GX/GY v- and r-pieces share base weights
            p2_srcs = [(P[VAL][0], 0), (P[CH_X][0], 1), (P[CH_Y][0], 2),
                       (P[CH_XT][0], 3), (P[CH_XT][1], 4),
                       (P[CH_YT][0], 5), (P[CH_YT][1], 6),
                       (P[CH_GX][0], 7), (P[CH_GX][1], 7),
                       (P[CH_GX][2], 8), (P[CH_GX][3], 9), (P[CH_GX][4], 10),
                       (P[CH_GY][0], 11), (P[CH_GY][1], 11),
                       (P[CH_GY][2], 12), (P[CH_GY][3], 13),
                       (P[CH_GY][4], 14)]
            for i, ((ap, _), wi) in enumerate(p2_srcs):
                mm(po2, o2w[wi], ap, start=(i == 0),
                   stop=(i == len(p2_srcs) - 1))

            a1 = o12.tile([12, NPT], f32, name="a1")
            act(a1[:, 0:n], po1t[0:12, 0:n], AF.Copy, bias=0.0, scale=1.0)
            a2 = o12.tile([12, NPT], f32, name="a2")
            act(a2[:, 0:n], po1t[32:44, 0:n], AF.Copy, bias=0.0, scale=1.0)
            pl1 = o12.tile([12, NPT], f32, name="pl1")
            tt(pl1[:, 0:n], a1[:, 0:n], po1t[64:76, 0:n], OP.mult)
            pl2 = o12.tile([12, NPT], f32, name="pl2")
            tt(pl2[:, 0:n], a2[:, 0:n], po1t[96:108, 0:n], OP.mult)
            dd = o12.tile([12, NPT], f32, name="dd")
            tt(dd[:, 0:n], pl1[:, 0:n], pl2[:, 0:n], OP.add)
            ff = o12.tile([12, NPT], f32, name="ff")
            stt(ff[:, 0:n], dd[:, 0:n], lam, po2t[96:108, 0:n],
                OP.mult, OP.add)
            pp = o6.tile([6, NPT], f32, name="pp")
            act(pp[:, 0:n], po2t[64:70, 0:n], AF.Copy,
                bias=float(p_bias), scale=1.0)
            uu = o6.tile([6, NPT], f32, name="uu")
            act(uu[:, 0:n], po2t[0:6, 0:n], AF.Copy, bias=0.0, scale=1.0)
            vv = o6.tile([6, NPT], f32, name="vv")
            act(vv[:, 0:n], po2t[32:38, 0:n], AF.Copy, bias=0.0, scale=1.0)

            dma(u_d[sidx][:, 0:n], uu[0:6, 0:n])
            dma(v_d[sidx][:, 0:n], vv[0:6, 0:n])
            dma(p_d[sidx][:, 0:n], pp[0:6, 0:n])
            dma(fu_d[sidx][:, 0:n], ff[0:6, 0:n])
            dma(fv_d[sidx][:, 0:n], ff[6:12, 0:n])

        def n_of(s):
            return NPT if s < FULL_STS else NL

        # l1 runs two supertiles ahead: its serial mm->tanh->DVE chain
        # fills the DVE holes at supertile boundaries
        Pq = {0: l1_block(0, n_of(0))}
        dma(wb[:], wB_d[:])
        if ns > 1:
            Pq[1] = l1_block(1, n_of(1))
        for s in range(ns):
            if s + 2 < ns:
                Pq[s + 2] = l1_block(s + 2, n_of(s + 2))
            body(s, Pq.pop(s), n_of(s))

    nc.compile()
    return nc


def make_in_maps(inputs, consts, ns=NS):
    x = np.asarray(inputs["x"], np.float32).reshape(-1)
    y = np.asarray(inputs["y"], np.float32).reshape(-1)
    t = np.asarray(inputs["t"], np.float32).reshape(-1)
    padpc = ns * SUPER
    shared = {k: consts[k] for k in ("wA", "wB", "cf")}
    in_maps = []
    for c in range(N_CORES):
        sl = slice(c * PPC, (c + 1) * PPC)

        def lay(vec):
            seg = vec[sl]
            out = np.zeros((ns, G, NPT), np.float32)
            out[:FULL_STS] = seg[:FULL_STS * SUPER].reshape(
                FULL_STS, G, NPT)
            last = np.zeros((G * NL,), np.float32)
            last[:REM] = seg[FULL_STS * SUPER:]
            out[FULL_STS, :, :NL] = last.reshape(G, NL)
            return out

        xyz = np.zeros((ns, 3 * G, NPT), np.float32)
        xyz[:, 0::3, :] = lay(x)
        xyz[:, 1::3, :] = lay(y)
        xyz[:, 2::3, :] = lay(t)
        xyz = np.ascontiguousarray(
            xyz.transpose(1, 0, 2).reshape(3 * G, ns * NPT))
        in_maps.append({"xyz": xyz.astype(np.float16), **shared})
    return in_maps


def unshard_core(a):
    """[NS, G, NPT] per-core output -> flat [PPC] in point order."""
    a = np.asarray(a).reshape(NS, G, NPT)
    head = a[:FULL_STS].reshape(-1)
    tail = a[FULL_STS, :, :NL].reshape(-1)[:REM]
    return np.concatenate([head, tail])


def kernel(**inputs):
    consts = build_host_consts(
        inputs["W_in"], inputs["b_in"], inputs["W_hid"], inputs["b_hid"],
        inputs["W_out"], inputs["b_out"], inputs["lb"], inputs["ub"],
        inputs["lambda_1"], inputs["lambda_2"])
    nc = build_program(consts["p_bias"])
    in_maps = make_in_maps(inputs, consts)

    from concourse.bass_utils import run_bass_kernel_spmd
    res = run_bass_kernel_spmd(nc, in_maps, list(range(N_CORES)))

    outs = []
    for name in ("u_out", "v_out", "p_out", "fu_out", "fv_out"):
        full = np.concatenate(
            [unshard_core(res.results[c][name])
             for c in range(N_CORES)])
        outs.append(np.ascontiguousarray(full[:, None], dtype=np.float32))
    return tuple(outs)

